# revision 12
# baseline (speedup 1.0000x reference)
"""Bass/Tile kernel v3 for BertUnpadSelfAttention on 8 TRN2 cores.

Problem shapes: B=4, S=1024, L=512 valid tokens/seq, H=12, D=64, DIM=768.
Sharding: core c handles batch b=c//2, heads h0=6*(c%2) .. h0+5.

Per-core pipeline (bf16 matmul data; bias fp8):
  warm:    dummy matmuls ramp the PE p-state while input DMAs land
  proj:    qkT[f, t] = wqkT.T @ xT  (q pre-scaled 1/8);  v[t, f] = xT.T @ wvT
           packed per head as v_aug [128, 6, 65] with ones col 64
  scores:  psum[kk, q] = kT_j.T @ qT_j + bias  (bias via fp8 identity matmul)
           ACT exp -> exp_v_j [128, 4, 512] bf16 (k-major)
  PV:      q-major ctx[qc][:, j, 0:65] += exp_v_j.T @ v_aug (col 64 = denom)
           plus a 1-col fp32 matmul adding the HOST-precomputed padded-key
           denominator den_p[q] into col 64 (padded keys have zero value
           rows, so only their exp(bias) sum matters - computed exactly on
           host, removing the biasp DMA + on-device exp entirely)
  norm:    rcp = 1/ctx[:, :, 64] (DVE, psum); out = ctx[:, :, 0:64] * rcp
           (broadcast along d) -> bf16 -> DMA
"""
import sys

sys.path.insert(0, "/opt/trn_rl_repo")

import numpy as np
import ml_dtypes

import concourse.bacc as bacc
import concourse.mybir as mybir
from concourse.tile import TileContext

F32 = mybir.dt.float32
BF16 = mybir.dt.bfloat16
FP8 = mybir.dt.float8e4
NP_BF16 = ml_dtypes.bfloat16
NP_FP8 = ml_dtypes.float8_e4m3
ALU = mybir.AluOpType
ACTF = mybir.ActivationFunctionType

P = 128
B, S, L = 4, 1024, 512
H, D = 12, 64
DIM = H * D
HPC = 6            # heads per core
T = 512            # tokens per core
QKF = 2 * HPC * D  # 768 q+k output features
VF = HPC * D       # 384 v output features
KC_IN = DIM // P   # 6 contraction chunks
NKC = L // P       # 4 valid-key chunks / q-chunks
NQC = 4
SCALE = 1.0 / 8.0
WARM_MMS = 9


def build_kernel(skip_qkv_bias=True):
    nc = bacc.Bacc("TRN2", target_bir_lowering=False, debug=False,
                   num_devices=8)

    xw = nc.dram_tensor("xw", [DIM, T + QKF + VF], BF16, kind="ExternalInput")
    biasv = nc.dram_tensor("biasv", [HPC, P, NKC, T], FP8,
                           kind="ExternalInput")
    denp = nc.dram_tensor("denp", [P, NQC, HPC], F32,
                          kind="ExternalInput")
    id8 = nc.dram_tensor("id8", [P, P], FP8, kind="ExternalInput")
    bqk = nc.dram_tensor("bqk", [P, KC_IN], F32, kind="ExternalInput")
    xob = nc.dram_tensor("xob", [1, T + VF], BF16, kind="ExternalInput")
    out = nc.dram_tensor("out", [NQC, P, HPC, D], BF16, kind="ExternalOutput")

    with TileContext(nc) as tc:
        with (
            tc.tile_pool(name="const", bufs=1) as cpool,
            tc.tile_pool(name="qkv", bufs=1) as qkvpool,
            tc.tile_pool(name="expv", bufs=1) as evpool,
            tc.tile_pool(name="outp", bufs=1) as opool,
        ):
            # ---- PE warm-up first: memset + dummy matmuls, no data deps ----
            warm_sb = cpool.tile([P, T], BF16, tag="warm")
            nc.vector.memset(warm_sb[:], 0.0)

            # ---- DMA issues: the sync HWDGE queue (q1) is capped at
            # ~30GB/s on this platform even running alone; scalar HWDGE
            # (~125GB/s) and gpsimd SWDGE (~143GB/s) are the fast queues.
            # Bulk input goes on scalar+gpsimd, outputs on gpsimd, sync
            # carries nothing bulk. Per-queue order matches consumption
            # order, ~2MB per queue.
            xw_sb = []
            for kc in range(KC_IN):
                t = cpool.tile([P, T + QKF + VF], BF16, tag=f"xw{kc}")
                xw_sb.append(t)
            biasv_sb = []
            for j in range(HPC):
                t = cpool.tile([P, NKC, T], FP8, tag=f"bv{j}")
                biasv_sb.append(t)
            id8_sb = cpool.tile([P, P], FP8, tag="id8")
            denp_sb = cpool.tile([P, NQC, HPC], F32, tag="denp")

            def qk_dma(eng, kc):
                eng.dma_start(out=xw_sb[kc][:, 0:T + QKF],
                              in_=xw[kc * P:(kc + 1) * P, 0:T + QKF])

            def vp_dma(eng, kc):
                eng.dma_start(out=xw_sb[kc][:, T + QKF:],
                              in_=xw[kc * P:(kc + 1) * P, T + QKF:])

            def bv_dma(eng, j):
                eng.dma_start(out=biasv_sb[j][:], in_=biasv[j])

            # gpsimd queue (fastest, ~178GB/s): 4 qk chunks + bias + vp
            qk_dma(nc.gpsimd, 0)
            qk_dma(nc.gpsimd, 1)
            qk_dma(nc.gpsimd, 3)
            bv_dma(nc.gpsimd, 0)
            bv_dma(nc.gpsimd, 2)
            bv_dma(nc.gpsimd, 4)
            vp_dma(nc.gpsimd, 3)
            vp_dma(nc.gpsimd, 5)
            # scalar queue (~132GB/s; first transfer delayed by the
            # framework ACT table load on the scalar engine)
            qk_dma(nc.scalar, 2)
            qk_dma(nc.scalar, 4)
            qk_dma(nc.scalar, 5)
            bv_dma(nc.scalar, 1)
            bv_dma(nc.scalar, 3)
            bv_dma(nc.scalar, 5)
            vp_dma(nc.scalar, 4)
            # sync queue (~30GB/s but idle): small consts + early v-weights
            nc.sync.dma_start(out=id8_sb[:], in_=id8[:])
            nc.sync.dma_start(out=denp_sb[:], in_=denp[:])
            vp_dma(nc.sync, 0)
            vp_dma(nc.sync, 1)
            vp_dma(nc.sync, 2)
            if not skip_qkv_bias:
                bqk_sb = cpool.tile([P, KC_IN], F32, tag="bqk")
                nc.sync.dma_start(out=bqk_sb[:], in_=bqk[:])
                xob_sb = cpool.tile([1, T + VF], BF16, tag="xob")
                nc.sync.dma_start(out=xob_sb[:], in_=xob[:])

            with tc.tile_pool(name="ps", bufs=1, space="PSUM") as ppool:
                # two 4-bank psum tiles rotate through roles: first
                # instance hosts the qk granules (+warm/fill targets),
                # later instances are per-head score tiles so each head
                # needs only ONE 2048-col ACT exp
                sA = ppool.tile([P, NKC, T], F32, tag="sA", name="sA")
                sB = ppool.tile([P, NKC, T], F32, tag="sB", name="sB")

                for wi in range(WARM_MMS):
                    nc.tensor.matmul(sB[:, 3, :], warm_sb[:, 0:P],
                                     warm_sb[:], start=True, stop=True)

                def fillers(n):
                    # dummy matmuls keep the PE busy (and its p-state high)
                    # while input chunks are still in flight
                    for _ in range(n):
                        nc.tensor.matmul(sB[:, 0, :], warm_sb[:, 0:P],
                                         warm_sb[:], start=True, stop=True)

                exp_v = [None] * HPC
                qkT_sb = [None] * KC_IN

                def wsl(mc, kc):
                    return xw_sb[kc][:, T + mc * P:T + (mc + 1) * P]

                # ---- qk projection, chunk-major over granules 0,3,1,4:
                # each x/w chunk is consumed across four granules right as
                # it lands, so the PE tracks DMA arrival with minimal fill.
                # KC_ORDER matches measured chunk arrival order.
                KC_ORDER = [2, 0, 4, 1, 3, 5]
                FILLS = {4: 1, 5: 2}
                GSLOT = [0, 3, 1, 4]
                for ki in range(KC_IN):
                    kc = KC_ORDER[ki]
                    if ki in FILLS:
                        fillers(FILLS[ki])
                    for gi, mc in enumerate(GSLOT):
                        nc.tensor.matmul(
                            sA[:, gi, :], wsl(mc, kc),
                            xw_sb[kc][:, 0:T],
                            start=(ki == 0), stop=(ki == KC_IN - 1),
                            skip_group_check=True)
                del kc

                def cast_qkT(mc, src):
                    qt = qkvpool.tile([P, T], BF16, tag=f"qkT{mc}",
                                      name=f"qkT{mc}")
                    if skip_qkv_bias:
                        nc.vector.tensor_copy(qt[:], src)
                    else:
                        nc.vector.tensor_scalar(
                            qt[:], src, bqk_sb[:, mc:mc + 1], None, ALU.add)
                    qkT_sb[mc] = qt

                # casts for the head-0/1 pair first so scores can start
                # while granules 2,5 still run on the PE
                cast_qkT(0, sA[:, 0, :])
                cast_qkT(3, sA[:, 1, :])

                # granules 2,5 (all chunks have landed by now)
                for hi, mc in enumerate([2, 5]):
                    for kc in range(KC_IN):
                        nc.tensor.matmul(
                            sB[:, 1 + hi, :], wsl(mc, kc),
                            xw_sb[kc][:, 0:T],
                            start=(kc == 0), stop=(kc == KC_IN - 1),
                            skip_group_check=True)

                cast_qkT(1, sA[:, 2, :])
                cast_qkT(4, sA[:, 3, :])
                cast_qkT(2, sB[:, 1, :])
                cast_qkT(5, sB[:, 2, :])

                v_sb = []
                for tch in range(NKC):
                    vt = qkvpool.tile([P, HPC, D + 1], BF16, tag=f"v{tch}",
                                      name=f"v{tch}")
                    nc.vector.memset(vt[:, :, D], 1.0)
                    v_sb.append(vt)

                # ---- scores + bias + one 2048-col exp per head; psum
                # tags sA/sB alternate between heads
                for j in range(HPC):
                    qT = qkT_sb[j // 2][(j % 2) * D:(j % 2) * D + D, :]
                    kTt = qkT_sb[3 + j // 2]
                    ev = evpool.tile([P, NKC, T], BF16, tag=f"ev{j}",
                                     name=f"ev{j}")
                    ps = ppool.tile([P, NKC, T], F32,
                                    tag=("sA" if j % 2 == 0 else "sB"),
                                    name=f"sc{j}")
                    for kc in range(NKC):
                        nc.tensor.matmul(
                            ps[:, kc, :],
                            kTt[(j % 2) * D:(j % 2) * D + D,
                                kc * P:(kc + 1) * P],
                            qT, start=True, stop=False)
                        nc.tensor.matmul(
                            ps[:, kc, :],
                            id8_sb[:],
                            biasv_sb[j][:, kc, :],
                            start=False, stop=True,
                            skip_group_check=True)
                    nc.scalar.activation(ev[:], ps[:], ACTF.Exp)
                    exp_v[j] = ev

                # ---- v projection ----
                for tcg in range(2):
                    vg = ppool.tile([P, NKC, T], F32,
                                    tag=("sA" if tcg == 0 else "sB"),
                                    name=f"vg{tcg}")
                    for hi in range(2):
                        tch = 2 * tcg + hi
                        psh = vg[:, hi, 0:VF]
                        for kc in range(KC_IN):
                            nc.tensor.matmul(
                                psh,
                                xw_sb[kc][:, tch * P:(tch + 1) * P],
                                xw_sb[kc][:, T + QKF:],
                                start=(kc == 0),
                                stop=(skip_qkv_bias and kc == KC_IN - 1))
                        if not skip_qkv_bias:
                            nc.tensor.matmul(
                                psh, xob_sb[:, tch * P:(tch + 1) * P],
                                xob_sb[:, T:], start=False, stop=True)
                        nc.vector.tensor_copy(
                            v_sb[tch][:, :, 0:D],
                            psh.rearrange("p (j d) -> p j d", j=HPC))

            # ---- PV head-outer so each head's exp is consumed as soon as
            # the ACT stream produces it; denominator adds the
            # host-precomputed padded-key sum; per-qc norm + out DMA ----
            with (
                tc.tile_pool(name="psc", bufs=1, space="PSUM") as ctxpool,
            ):
                ctxs = [ctxpool.tile([P, HPC, D + 1], F32, tag=f"c{qc}",
                                     name=f"c{qc}")
                        for qc in range(NQC)]
                for j in range(HPC - 1):
                    for qc in range(NQC):
                        ctx = ctxs[qc]
                        for kc in range(NKC):
                            nc.tensor.matmul(
                                ctx[:, j, :],
                                exp_v[j][:, kc, qc * P:(qc + 1) * P],
                                v_sb[kc][:, j, :],
                                start=(kc == 0), stop=(kc == NKC - 1),
                                skip_group_check=True)
                # last head's PV is interleaved with the per-qc norm + out
                # so each q-chunk drains as soon as its accumulation stops
                for qc in range(NQC):
                    ctx = ctxs[qc]
                    j = HPC - 1
                    for kc in range(NKC):
                        nc.tensor.matmul(
                            ctx[:, j, :],
                            exp_v[j][:, kc, qc * P:(qc + 1) * P],
                            v_sb[kc][:, j, :],
                            start=(kc == 0), stop=(kc == NKC - 1),
                            skip_group_check=True)
                    dsum = opool.tile([P, HPC], F32, tag=f"ds{qc}",
                                      name=f"ds{qc}")
                    nc.vector.tensor_tensor(
                        out=dsum[:], in0=ctx[:, :, D],
                        in1=denp_sb[:, qc, :], op=ALU.add)
                    rcp = opool.tile([P, HPC], F32, tag=f"rcp{qc}",
                                     name=f"rcp{qc}")
                    nc.vector.reciprocal(rcp[:], dsum[:])
                    ot = opool.tile([P, HPC, D], BF16, tag=f"ot{qc}",
                                    name=f"ot{qc}")
                    nc.vector.tensor_tensor(
                        out=ot[:], in0=ctx[:, :, 0:D],
                        in1=rcp[:, :, None].broadcast_to([P, HPC, D]),
                        op=ALU.mult)
                    (nc.scalar if qc % 2 == 0 else nc.gpsimd).dma_start(
                        out=out[qc], in_=ot[:])

    nc.compile()
    return nc


# ---------------- host-side sharding ----------------

def make_core_inputs(hidden_states, Wqkv_w, Wqkv_b, bias, core):
    b, half = core // 2, core % 2
    h0 = HPC * half
    xT = np.ascontiguousarray(hidden_states[b * T:(b + 1) * T, :].T)
    wq = Wqkv_w[h0 * D:(h0 + HPC) * D, :] * np.float32(SCALE)
    wk = Wqkv_w[DIM + h0 * D:DIM + (h0 + HPC) * D, :]
    wv = Wqkv_w[2 * DIM + h0 * D:2 * DIM + (h0 + HPC) * D, :]
    wqkT = np.concatenate([wq, wk], axis=0).T
    wvT = wv.T
    xwc = np.concatenate([xT, wqkT, wvT], axis=1).astype(NP_BF16)

    # k-major bias: [j, p, kc, q] = bias[b, h0+j, q, kc*128 + p]
    bt = bias[b, h0:h0 + HPC, :T, :]                   # (j, q, k)
    bv = bt[:, :, :L].transpose(0, 2, 1)               # (j, k, q) valid
    biasv = np.ascontiguousarray(
        bv.reshape(HPC, NKC, P, T).transpose(0, 2, 1, 3)).astype(NP_FP8)
    # padded-key denominator: den_p[j, q] = sum_k exp(bias[j, q, k>=L]),
    # computed exactly on host (padded keys contribute zero value rows)
    bp = bt[:, :, L:].astype(np.float32)                # (j, q, k')
    den_p = np.exp(bp).sum(axis=2)                      # (j, q)
    denp = np.ascontiguousarray(
        den_p.T.reshape(NQC, P, HPC).transpose(1, 0, 2)
    ).astype(np.float32)                                # [p, qc, j]

    bq = Wqkv_b[h0 * D:(h0 + HPC) * D] * np.float32(SCALE)
    bk = Wqkv_b[DIM + h0 * D:DIM + (h0 + HPC) * D]
    bqk = np.concatenate([bq, bk]).reshape(KC_IN, P).T  # [128, 6]
    bv_ = Wqkv_b[2 * DIM + h0 * D:2 * DIM + (h0 + HPC) * D]
    xob = np.concatenate([np.ones(T, np.float32), bv_])[None, :]

    return dict(
        xw=xwc,
        biasv=biasv,
        denp=denp,
        id8=np.eye(P, dtype=np.float32).astype(NP_FP8),
        bqk=np.ascontiguousarray(bqk).astype(np.float32),
        xob=xob.astype(NP_BF16),
    )


def assemble_output(core_outs):
    full = np.empty((B * T, DIM), np.float32)
    for core, arr in enumerate(core_outs):
        b, half = core // 2, core % 2
        h0 = HPC * half
        full[b * T:(b + 1) * T, h0 * D:(h0 + HPC) * D] = (
            np.asarray(arr).astype(np.float32).reshape(T, HPC * D))
    return full


def core_reference(ci):
    """numpy reference of the per-core shard -> (NQC, P, HPC, D)."""
    xw_ = np.asarray(ci["xw"]).astype(np.float32)
    xT_ = xw_[:, 0:T]
    qkT = xw_[:, T:T + QKF].T @ xT_
    v = xT_.T @ xw_[:, T + QKF:]
    bqk = np.asarray(ci["bqk"]).astype(np.float32).T.reshape(-1)
    qkT = qkT + bqk[:, None]
    xob = np.asarray(ci["xob"]).astype(np.float32)[0]
    v = v + xob[T:][None, :]
    biasv = np.asarray(ci["biasv"]).astype(np.float32)  # [j, p, kc, q]
    denp = np.asarray(ci["denp"]).astype(np.float32)    # [p, qc, j]
    outs = np.zeros((NQC, P, HPC, D), np.float32)
    for j in range(HPC):
        qT = qkT[j * D:(j + 1) * D, :]
        kT = qkT[VF + j * D:VF + (j + 1) * D, :]
        bt = biasv[j].transpose(1, 0, 2).reshape(L, T)   # [k, q]
        st = kT.T @ qT + bt
        ep_v = np.exp(st)
        den_p = denp[:, :, j].T.reshape(T)               # [q]
        vh = v[:, j * D:(j + 1) * D]
        ctx = ep_v.T @ vh                                # [q, d]
        den = ep_v.sum(0) + den_p
        o = ctx / den[:, None]
        outs[:, :, j, :] = o.reshape(NQC, P, D)
    return outs


# ---------------- public entry point ----------------

_NC_CACHE = {}


def _get_nc(skip_qkv_bias):
    if skip_qkv_bias not in _NC_CACHE:
        _NC_CACHE[skip_qkv_bias] = build_kernel(skip_qkv_bias=skip_qkv_bias)
    return _NC_CACHE[skip_qkv_bias]


def _canonical(hidden_states, Wqkv_w, Wqkv_b, bias, indices, attn_mask,
               cu_seqlens, max_seqlen_in_batch):
    if hidden_states.shape != (B * T, DIM) or Wqkv_w.shape != (3 * DIM, DIM):
        return False
    if bias.shape != (B, H, S, S) or indices.shape != (B * T,):
        return False
    if int(max_seqlen_in_batch) != S or attn_mask.shape != (B, S):
        return False
    want = (np.arange(B)[:, None] * S + np.arange(T)[None, :]).reshape(-1)
    return bool((indices.astype(np.int64) == want).all())


def _reference_fallback(hidden_states, Wqkv_w, Wqkv_b, bias, indices,
                        attn_mask, cu_seqlens, max_seqlen_in_batch):
    b = attn_mask.shape[0]
    s = int(max_seqlen_in_batch)
    h = bias.shape[1]
    d = Wqkv_w.shape[1] // h
    qkv = hidden_states.astype(np.float32) @ Wqkv_w.astype(np.float32).T
    qkv = qkv + Wqkv_b.astype(np.float32)
    padded = np.zeros((b * s, qkv.shape[-1]), np.float32)
    padded[indices.astype(np.int64)] = qkv
    qkv = padded.reshape(b, s, 3, h, d)
    q, k, v = qkv[:, :, 0], qkv[:, :, 1], qkv[:, :, 2]
    scale = 1.0 / float(np.sqrt(d))
    scores = np.einsum("bqhd,bkhd->bhqk", q, k) * scale
    scores = scores + bias.astype(np.float32)
    scores -= scores.max(axis=-1, keepdims=True)
    probs = np.exp(scores)
    probs /= probs.sum(axis=-1, keepdims=True)
    ctx = np.einsum("bhqk,bkhd->bqhd", probs, v)
    return ctx.reshape(b * s, h * d)[indices.astype(np.int64)].astype(
        np.float32)


def kernel(hidden_states, Wqkv_w, Wqkv_b, bias, indices, attn_mask,
           cu_seqlens, max_seqlen_in_batch):
    hidden_states = np.asarray(hidden_states)
    Wqkv_w = np.asarray(Wqkv_w)
    Wqkv_b = np.asarray(Wqkv_b)
    bias = np.asarray(bias)
    indices = np.asarray(indices)
    attn_mask = np.asarray(attn_mask)

    if not _canonical(hidden_states, Wqkv_w, Wqkv_b, bias, indices,
                      attn_mask, cu_seqlens, max_seqlen_in_batch):
        return _reference_fallback(hidden_states, Wqkv_w, Wqkv_b, bias,
                                   indices, attn_mask, cu_seqlens,
                                   max_seqlen_in_batch)

    from concourse.bass_utils import run_bass_kernel_spmd

    skip_bias = bool((Wqkv_b == 0).all())
    nc = _get_nc(skip_bias)
    in_maps = [
        make_core_inputs(hidden_states, Wqkv_w, Wqkv_b, bias, core)
        for core in range(8)
    ]
    out = None
    for _ in range(4):
        res = run_bass_kernel_spmd(nc, in_maps, list(range(8)))
        out = assemble_output([res.results[c]["out"] for c in range(8)])
        # softmax-averaged values are bounded ~O(1); device-fault garbage is
        # astronomically larger - rerun if detected
        if np.isfinite(out).all() and np.abs(out).max() < 10.0:
            break
    return out


# revision 13
# speedup vs baseline: 1.0725x; 1.0725x over previous
"""Bass/Tile kernel v3 for BertUnpadSelfAttention on 8 TRN2 cores.

Problem shapes: B=4, S=1024, L=512 valid tokens/seq, H=12, D=64, DIM=768.
Sharding: core c handles batch b=c//2, heads h0=6*(c%2) .. h0+5.

Per-core pipeline (bf16 matmul data; bias fp8):
  warm:    dummy matmuls ramp the PE p-state while input DMAs land
  proj:    qkT[f, t] = wqkT.T @ xT  (q pre-scaled 1/8);  v[t, f] = xT.T @ wvT
           packed per head as v_aug [128, 6, 65] with ones col 64
  scores:  psum[kk, q] = kT_j.T @ qT_j + bias  (bias via fp8 identity matmul)
           ACT exp -> exp_v_j [128, 4, 512] bf16 (k-major)
  PV:      q-major ctx[qc][:, j, 0:65] += exp_v_j.T @ v_aug (col 64 = denom)
           plus a 1-col fp32 matmul adding the HOST-precomputed padded-key
           denominator den_p[q] into col 64 (padded keys have zero value
           rows, so only their exp(bias) sum matters - computed exactly on
           host, removing the biasp DMA + on-device exp entirely)
  norm:    rcp = 1/ctx[:, :, 64] (DVE, psum); out = ctx[:, :, 0:64] * rcp
           (broadcast along d) -> bf16 -> DMA
"""
import sys

sys.path.insert(0, "/opt/trn_rl_repo")

import numpy as np
import ml_dtypes

import concourse.bacc as bacc
import concourse.mybir as mybir
from concourse.tile import TileContext

F32 = mybir.dt.float32
BF16 = mybir.dt.bfloat16
FP8 = mybir.dt.float8e4
NP_BF16 = ml_dtypes.bfloat16
NP_FP8 = ml_dtypes.float8_e4m3
ALU = mybir.AluOpType
ACTF = mybir.ActivationFunctionType

P = 128
B, S, L = 4, 1024, 512
H, D = 12, 64
DIM = H * D
HPC = 6            # heads per core
T = 512            # tokens per core
QKF = 2 * HPC * D  # 768 q+k output features
VF = HPC * D       # 384 v output features
KC_IN = DIM // P   # 6 contraction chunks
NKC = L // P       # 4 valid-key chunks / q-chunks
NQC = 4
SCALE = 1.0 / 8.0
WARM_MMS = 9


def build_kernel(skip_qkv_bias=True):
    nc = bacc.Bacc("TRN2", target_bir_lowering=False, debug=False,
                   num_devices=8)

    xw = nc.dram_tensor("xw", [DIM, T + QKF + VF], BF16, kind="ExternalInput")
    biasv = nc.dram_tensor("biasv", [HPC, P, NKC, T], FP8,
                           kind="ExternalInput")
    denp = nc.dram_tensor("denp", [P, NQC, HPC], F32,
                          kind="ExternalInput")
    id8 = nc.dram_tensor("id8", [P, P], FP8, kind="ExternalInput")
    bqk = nc.dram_tensor("bqk", [P, KC_IN], F32, kind="ExternalInput")
    xob = nc.dram_tensor("xob", [1, T + VF], BF16, kind="ExternalInput")
    out = nc.dram_tensor("out", [NQC, P, HPC, D], BF16, kind="ExternalOutput")

    with TileContext(nc) as tc:
        with (
            tc.tile_pool(name="const", bufs=1) as cpool,
            tc.tile_pool(name="qkv", bufs=1) as qkvpool,
            tc.tile_pool(name="expv", bufs=1) as evpool,
            tc.tile_pool(name="outp", bufs=1) as opool,
        ):
            # ---- PE warm-up first: memset + dummy matmuls, no data deps ----
            warm_sb = cpool.tile([P, T], BF16, tag="warm")
            nc.vector.memset(warm_sb[:], 0.0)

            # ---- DMA issues: the sync HWDGE queue (q1) is capped at
            # ~30GB/s on this platform even running alone; scalar HWDGE
            # (~125GB/s) and gpsimd SWDGE (~143GB/s) are the fast queues.
            # Bulk input goes on scalar+gpsimd, outputs on gpsimd, sync
            # carries nothing bulk. Per-queue order matches consumption
            # order, ~2MB per queue.
            xw_sb = []
            for kc in range(KC_IN):
                t = cpool.tile([P, T + QKF + VF], BF16, tag=f"xw{kc}")
                xw_sb.append(t)
            biasv_sb = []
            for j in range(HPC):
                t = cpool.tile([P, NKC, T], FP8, tag=f"bv{j}")
                biasv_sb.append(t)
            id8_sb = cpool.tile([P, P], FP8, tag="id8")
            denp_sb = cpool.tile([P, NQC, HPC], F32, tag="denp")

            def qk_dma(eng, kc):
                eng.dma_start(out=xw_sb[kc][:, 0:T + QKF],
                              in_=xw[kc * P:(kc + 1) * P, 0:T + QKF])

            def vp_dma(eng, kc):
                eng.dma_start(out=xw_sb[kc][:, T + QKF:],
                              in_=xw[kc * P:(kc + 1) * P, T + QKF:])

            def bv_dma(eng, j):
                eng.dma_start(out=biasv_sb[j][:], in_=biasv[j])

            # gpsimd queue (fastest, ~178GB/s): 4 qk chunks + bias + vp
            qk_dma(nc.gpsimd, 0)
            qk_dma(nc.gpsimd, 1)
            qk_dma(nc.gpsimd, 3)
            bv_dma(nc.gpsimd, 0)
            bv_dma(nc.gpsimd, 2)
            bv_dma(nc.gpsimd, 4)
            vp_dma(nc.gpsimd, 3)
            vp_dma(nc.gpsimd, 5)
            # scalar queue (~132GB/s; first transfer delayed by the
            # framework ACT table load on the scalar engine)
            qk_dma(nc.scalar, 2)
            qk_dma(nc.scalar, 4)
            qk_dma(nc.scalar, 5)
            bv_dma(nc.scalar, 1)
            bv_dma(nc.scalar, 3)
            bv_dma(nc.scalar, 5)
            vp_dma(nc.scalar, 4)
            # sync queue (~30GB/s but idle): small consts + early v-weights
            nc.sync.dma_start(out=id8_sb[:], in_=id8[:])
            nc.sync.dma_start(out=denp_sb[:], in_=denp[:])
            vp_dma(nc.sync, 0)
            vp_dma(nc.sync, 1)
            vp_dma(nc.sync, 2)
            if not skip_qkv_bias:
                bqk_sb = cpool.tile([P, KC_IN], F32, tag="bqk")
                nc.sync.dma_start(out=bqk_sb[:], in_=bqk[:])
                xob_sb = cpool.tile([1, T + VF], BF16, tag="xob")
                nc.sync.dma_start(out=xob_sb[:], in_=xob[:])

            with tc.tile_pool(name="ps", bufs=1, space="PSUM") as ppool:
                # four 2-bank psum tiles: instance 1 of s1..s3 hosts the six
                # qk granule chains (chunk-major), s0 takes warm/fill
                # dummies; later instances rotate as half-head score tiles
                # (4-deep rotation keeps the PE ahead of the ACT exp stream)
                sp = [ppool.tile([P, 2, T], F32, tag=f"s{i}", name=f"s{i}")
                      for i in range(4)]

                for wi in range(WARM_MMS):
                    nc.tensor.matmul(sp[0][:, 0, :], warm_sb[:, 0:P],
                                     warm_sb[:], start=True, stop=True)

                def fillers(n):
                    # dummy matmuls keep the PE busy (and its p-state high)
                    # while input chunks are still in flight
                    for _ in range(n):
                        nc.tensor.matmul(sp[0][:, 1, :], warm_sb[:, 0:P],
                                         warm_sb[:], start=True, stop=True)

                exp_v = [None] * HPC
                qkT_sb = [None] * KC_IN

                def wsl(mc, kc):
                    return xw_sb[kc][:, T + mc * P:T + (mc + 1) * P]

                # ---- qk projection, chunk-major over ALL six granules:
                # each x/w chunk is consumed across the six granule chains
                # right as it lands, so every granule completes with the
                # last chunk. KC_ORDER matches measured arrival order.
                KC_ORDER = [2, 0, 4, 1, 3, 5]
                FILLS = {4: 1, 5: 2}
                GSLOT = [(1, 0, 0), (1, 1, 3), (2, 0, 1),
                         (2, 1, 4), (3, 0, 2), (3, 1, 5)]
                for ki in range(KC_IN):
                    kc = KC_ORDER[ki]
                    if ki in FILLS:
                        fillers(FILLS[ki])
                    for ti, half, mc in GSLOT:
                        nc.tensor.matmul(
                            sp[ti][:, half, :], wsl(mc, kc),
                            xw_sb[kc][:, 0:T],
                            start=(ki == 0), stop=(ki == KC_IN - 1),
                            skip_group_check=True)
                del kc

                def cast_qkT(mc, src):
                    qt = qkvpool.tile([P, T], BF16, tag=f"qkT{mc}",
                                      name=f"qkT{mc}")
                    if skip_qkv_bias:
                        nc.vector.tensor_copy(qt[:], src)
                    else:
                        nc.vector.tensor_scalar(
                            qt[:], src, bqk_sb[:, mc:mc + 1], None, ALU.add)
                    qkT_sb[mc] = qt

                # head-pair (0,1) casts first so their scores start while
                # the remaining casts drain on the DVE
                cast_qkT(0, sp[1][:, 0, :])
                cast_qkT(3, sp[1][:, 1, :])
                cast_qkT(1, sp[2][:, 0, :])
                cast_qkT(4, sp[2][:, 1, :])
                cast_qkT(2, sp[3][:, 0, :])
                cast_qkT(5, sp[3][:, 1, :])

                v_sb = []
                for tch in range(NKC):
                    vt = qkvpool.tile([P, HPC, D + 1], BF16, tag=f"v{tch}",
                                      name=f"v{tch}")
                    nc.vector.memset(vt[:, :, D], 1.0)
                    v_sb.append(vt)

                # ---- scores + bias + exp per half-head; psum tags rotate
                # s0,s1,s2,s3 giving a 4-deep PE-ahead-of-ACT pipeline
                for j in range(HPC):
                    qT = qkT_sb[j // 2][(j % 2) * D:(j % 2) * D + D, :]
                    kTt = qkT_sb[3 + j // 2]
                    ev = evpool.tile([P, NKC, T], BF16, tag=f"ev{j}",
                                     name=f"ev{j}")
                    for pc in range(2):
                        ps = ppool.tile([P, 2, T], F32,
                                        tag=f"s{(2 * j + pc) % 4}",
                                        name=f"sc{j}_{pc}")
                        for i in range(2):
                            kc = 2 * pc + i
                            nc.tensor.matmul(
                                ps[:, i, :],
                                kTt[(j % 2) * D:(j % 2) * D + D,
                                    kc * P:(kc + 1) * P],
                                qT, start=True, stop=False)
                        for i in range(2):
                            kc = 2 * pc + i
                            nc.tensor.matmul(
                                ps[:, i, :],
                                id8_sb[:],
                                biasv_sb[j][:, kc, :],
                                start=False, stop=True,
                                skip_group_check=True)
                        nc.scalar.activation(
                            ev[:, 2 * pc:2 * pc + 2, :], ps[:], ACTF.Exp)
                    exp_v[j] = ev

                # ---- v projection ----
                for tcg in range(2):
                    vg = ppool.tile([P, 2, T], F32, tag=f"s{tcg}",
                                    name=f"vg{tcg}")
                    for hi in range(2):
                        tch = 2 * tcg + hi
                        psh = vg[:, hi, 0:VF]
                        for kc in range(KC_IN):
                            nc.tensor.matmul(
                                psh,
                                xw_sb[kc][:, tch * P:(tch + 1) * P],
                                xw_sb[kc][:, T + QKF:],
                                start=(kc == 0),
                                stop=(skip_qkv_bias and kc == KC_IN - 1))
                        if not skip_qkv_bias:
                            nc.tensor.matmul(
                                psh, xob_sb[:, tch * P:(tch + 1) * P],
                                xob_sb[:, T:], start=False, stop=True)
                        nc.vector.tensor_copy(
                            v_sb[tch][:, :, 0:D],
                            psh.rearrange("p (j d) -> p j d", j=HPC))

            # ---- PV head-outer so each head's exp is consumed as soon as
            # the ACT stream produces it; denominator adds the
            # host-precomputed padded-key sum; per-qc norm + out DMA ----
            with (
                tc.tile_pool(name="psc", bufs=1, space="PSUM") as ctxpool,
            ):
                ctxs = [ctxpool.tile([P, HPC, D + 1], F32, tag=f"c{qc}",
                                     name=f"c{qc}")
                        for qc in range(NQC)]
                for j in range(HPC - 1):
                    for qc in range(NQC):
                        ctx = ctxs[qc]
                        for kc in range(NKC):
                            nc.tensor.matmul(
                                ctx[:, j, :],
                                exp_v[j][:, kc, qc * P:(qc + 1) * P],
                                v_sb[kc][:, j, :],
                                start=(kc == 0), stop=(kc == NKC - 1),
                                skip_group_check=True)
                # last head's PV is interleaved with the per-qc norm + out
                # so each q-chunk drains as soon as its accumulation stops
                for qc in range(NQC):
                    ctx = ctxs[qc]
                    j = HPC - 1
                    for kc in range(NKC):
                        nc.tensor.matmul(
                            ctx[:, j, :],
                            exp_v[j][:, kc, qc * P:(qc + 1) * P],
                            v_sb[kc][:, j, :],
                            start=(kc == 0), stop=(kc == NKC - 1),
                            skip_group_check=True)
                    dsum = opool.tile([P, HPC], F32, tag=f"ds{qc}",
                                      name=f"ds{qc}")
                    nc.vector.tensor_tensor(
                        out=dsum[:], in0=ctx[:, :, D],
                        in1=denp_sb[:, qc, :], op=ALU.add)
                    rcp = opool.tile([P, HPC], F32, tag=f"rcp{qc}",
                                     name=f"rcp{qc}")
                    nc.vector.reciprocal(rcp[:], dsum[:])
                    ot = opool.tile([P, HPC, D], BF16, tag=f"ot{qc}",
                                    name=f"ot{qc}")
                    nc.vector.tensor_tensor(
                        out=ot[:], in0=ctx[:, :, 0:D],
                        in1=rcp[:, :, None].broadcast_to([P, HPC, D]),
                        op=ALU.mult)
                    (nc.scalar if qc % 2 == 0 else nc.gpsimd).dma_start(
                        out=out[qc], in_=ot[:])

    nc.compile()
    return nc


# ---------------- host-side sharding ----------------

def make_core_inputs(hidden_states, Wqkv_w, Wqkv_b, bias, core):
    b, half = core // 2, core % 2
    h0 = HPC * half
    xT = np.ascontiguousarray(hidden_states[b * T:(b + 1) * T, :].T)
    wq = Wqkv_w[h0 * D:(h0 + HPC) * D, :] * np.float32(SCALE)
    wk = Wqkv_w[DIM + h0 * D:DIM + (h0 + HPC) * D, :]
    wv = Wqkv_w[2 * DIM + h0 * D:2 * DIM + (h0 + HPC) * D, :]
    wqkT = np.concatenate([wq, wk], axis=0).T
    wvT = wv.T
    xwc = np.concatenate([xT, wqkT, wvT], axis=1).astype(NP_BF16)

    # k-major bias: [j, p, kc, q] = bias[b, h0+j, q, kc*128 + p]
    bt = bias[b, h0:h0 + HPC, :T, :]                   # (j, q, k)
    bv = bt[:, :, :L].transpose(0, 2, 1)               # (j, k, q) valid
    biasv = np.ascontiguousarray(
        bv.reshape(HPC, NKC, P, T).transpose(0, 2, 1, 3)).astype(NP_FP8)
    # padded-key denominator: den_p[j, q] = sum_k exp(bias[j, q, k>=L]),
    # computed exactly on host (padded keys contribute zero value rows)
    bp = bt[:, :, L:].astype(np.float32)                # (j, q, k')
    den_p = np.exp(bp).sum(axis=2)                      # (j, q)
    denp = np.ascontiguousarray(
        den_p.T.reshape(NQC, P, HPC).transpose(1, 0, 2)
    ).astype(np.float32)                                # [p, qc, j]

    bq = Wqkv_b[h0 * D:(h0 + HPC) * D] * np.float32(SCALE)
    bk = Wqkv_b[DIM + h0 * D:DIM + (h0 + HPC) * D]
    bqk = np.concatenate([bq, bk]).reshape(KC_IN, P).T  # [128, 6]
    bv_ = Wqkv_b[2 * DIM + h0 * D:2 * DIM + (h0 + HPC) * D]
    xob = np.concatenate([np.ones(T, np.float32), bv_])[None, :]

    return dict(
        xw=xwc,
        biasv=biasv,
        denp=denp,
        id8=np.eye(P, dtype=np.float32).astype(NP_FP8),
        bqk=np.ascontiguousarray(bqk).astype(np.float32),
        xob=xob.astype(NP_BF16),
    )


def assemble_output(core_outs):
    full = np.empty((B * T, DIM), np.float32)
    for core, arr in enumerate(core_outs):
        b, half = core // 2, core % 2
        h0 = HPC * half
        full[b * T:(b + 1) * T, h0 * D:(h0 + HPC) * D] = (
            np.asarray(arr).astype(np.float32).reshape(T, HPC * D))
    return full


def core_reference(ci):
    """numpy reference of the per-core shard -> (NQC, P, HPC, D)."""
    xw_ = np.asarray(ci["xw"]).astype(np.float32)
    xT_ = xw_[:, 0:T]
    qkT = xw_[:, T:T + QKF].T @ xT_
    v = xT_.T @ xw_[:, T + QKF:]
    bqk = np.asarray(ci["bqk"]).astype(np.float32).T.reshape(-1)
    qkT = qkT + bqk[:, None]
    xob = np.asarray(ci["xob"]).astype(np.float32)[0]
    v = v + xob[T:][None, :]
    biasv = np.asarray(ci["biasv"]).astype(np.float32)  # [j, p, kc, q]
    denp = np.asarray(ci["denp"]).astype(np.float32)    # [p, qc, j]
    outs = np.zeros((NQC, P, HPC, D), np.float32)
    for j in range(HPC):
        qT = qkT[j * D:(j + 1) * D, :]
        kT = qkT[VF + j * D:VF + (j + 1) * D, :]
        bt = biasv[j].transpose(1, 0, 2).reshape(L, T)   # [k, q]
        st = kT.T @ qT + bt
        ep_v = np.exp(st)
        den_p = denp[:, :, j].T.reshape(T)               # [q]
        vh = v[:, j * D:(j + 1) * D]
        ctx = ep_v.T @ vh                                # [q, d]
        den = ep_v.sum(0) + den_p
        o = ctx / den[:, None]
        outs[:, :, j, :] = o.reshape(NQC, P, D)
    return outs


# ---------------- public entry point ----------------

_NC_CACHE = {}


def _get_nc(skip_qkv_bias):
    if skip_qkv_bias not in _NC_CACHE:
        _NC_CACHE[skip_qkv_bias] = build_kernel(skip_qkv_bias=skip_qkv_bias)
    return _NC_CACHE[skip_qkv_bias]


def _canonical(hidden_states, Wqkv_w, Wqkv_b, bias, indices, attn_mask,
               cu_seqlens, max_seqlen_in_batch):
    if hidden_states.shape != (B * T, DIM) or Wqkv_w.shape != (3 * DIM, DIM):
        return False
    if bias.shape != (B, H, S, S) or indices.shape != (B * T,):
        return False
    if int(max_seqlen_in_batch) != S or attn_mask.shape != (B, S):
        return False
    want = (np.arange(B)[:, None] * S + np.arange(T)[None, :]).reshape(-1)
    return bool((indices.astype(np.int64) == want).all())


def _reference_fallback(hidden_states, Wqkv_w, Wqkv_b, bias, indices,
                        attn_mask, cu_seqlens, max_seqlen_in_batch):
    b = attn_mask.shape[0]
    s = int(max_seqlen_in_batch)
    h = bias.shape[1]
    d = Wqkv_w.shape[1] // h
    qkv = hidden_states.astype(np.float32) @ Wqkv_w.astype(np.float32).T
    qkv = qkv + Wqkv_b.astype(np.float32)
    padded = np.zeros((b * s, qkv.shape[-1]), np.float32)
    padded[indices.astype(np.int64)] = qkv
    qkv = padded.reshape(b, s, 3, h, d)
    q, k, v = qkv[:, :, 0], qkv[:, :, 1], qkv[:, :, 2]
    scale = 1.0 / float(np.sqrt(d))
    scores = np.einsum("bqhd,bkhd->bhqk", q, k) * scale
    scores = scores + bias.astype(np.float32)
    scores -= scores.max(axis=-1, keepdims=True)
    probs = np.exp(scores)
    probs /= probs.sum(axis=-1, keepdims=True)
    ctx = np.einsum("bhqk,bkhd->bqhd", probs, v)
    return ctx.reshape(b * s, h * d)[indices.astype(np.int64)].astype(
        np.float32)


def kernel(hidden_states, Wqkv_w, Wqkv_b, bias, indices, attn_mask,
           cu_seqlens, max_seqlen_in_batch):
    hidden_states = np.asarray(hidden_states)
    Wqkv_w = np.asarray(Wqkv_w)
    Wqkv_b = np.asarray(Wqkv_b)
    bias = np.asarray(bias)
    indices = np.asarray(indices)
    attn_mask = np.asarray(attn_mask)

    if not _canonical(hidden_states, Wqkv_w, Wqkv_b, bias, indices,
                      attn_mask, cu_seqlens, max_seqlen_in_batch):
        return _reference_fallback(hidden_states, Wqkv_w, Wqkv_b, bias,
                                   indices, attn_mask, cu_seqlens,
                                   max_seqlen_in_batch)

    from concourse.bass_utils import run_bass_kernel_spmd

    skip_bias = bool((Wqkv_b == 0).all())
    nc = _get_nc(skip_bias)
    in_maps = [
        make_core_inputs(hidden_states, Wqkv_w, Wqkv_b, bias, core)
        for core in range(8)
    ]
    out = None
    for _ in range(4):
        res = run_bass_kernel_spmd(nc, in_maps, list(range(8)))
        out = assemble_output([res.results[c]["out"] for c in range(8)])
        # softmax-averaged values are bounded ~O(1); device-fault garbage is
        # astronomically larger - rerun if detected
        if np.isfinite(out).all() and np.abs(out).max() < 10.0:
            break
    return out
